# revision 44
# baseline (speedup 1.0000x reference)
"""Causal self-attention with RoPE, fused Trainium2 Bass kernel, 8 NeuronCores.

Problem: x[4,2048,1024] @ W_attn[1024,3072] -> qkv; RoPE(q,k); causal
softmax attention (16 heads, d=64); y @ W_proj[1024,1024].

Sharding (data + head parallel): core c handles batch b=c//2 and heads
8*(c%2)..8*(c%2)+7.  W_attn is column-sharded by head, W_proj row-sharded;
each core emits a partial output projection and the host sums the two
partials per batch (the 2-way "all-reduce").

Kernel layout choices (per core):
 - Everything transposed: xT [D,T] in SBUF, q/k produced as qT/kT [d,T],
   attention computed as scoresT [k,q] so softmax-sum and PV contraction
   both run along the partition axis via matmuls (no transposes needed).
 - RoPE: head-dim channels are pre-permuted (via W_attn column permutation)
   into [e0..e15, o0..o15, e16..e31, o16..o31] per head so the rotate-half
   pairing is a 16<->16 swap inside each 32-partition quadrant, done with a
   single DVE stream_shuffle.  cos/sin tables (sign-folded) come from host.
 - No max-subtraction in softmax: scores/8 are ~N(0,0.4), exp is safe.
   l (row sum) comes free by appending a ones column to V (M=65 PV matmul).
 - fp16 operands everywhere on the PE (full-rate); fp32 accumulation.
 - Scores matmuls for a head pair run concurrently via row-tiled PE
   (stationaries at base partitions 0/64, separate PSUM banks).
 - Normalization: yt-releasing copies first on DVE, then 1/l (DVE),
   partition-broadcast (GPSIMD, both writes at base partition 0), and an
   in-place DVE multiply.  The chain only gates the pr3 out-proj riders.
 - Inputs arrive as a few large fully-contiguous DMAs (host pre-packs
   the SBUF layouts); outputs are f16 partials in per-DMA-contiguous
   blocks summed/transposed on host.
 - Scheduling: HAM warmup matmuls + exp-table preload burn the initial
   DMA wait; kb's run in pairs (scores 2j,2j+1 adjacent, PV lag-2) to
   halve the PE's 64/128-row reconfig penalty; the qk-projection and
   out-proj units are chopped into 2-matmul micro-steps and trickled
   into the attention loop against per-unit deadlines, keeping the PE
   dense while ACT streams the exps; the last out-proj units run in a
   4-deep PSUM pool so only their pair-3 matmuls wait on the final
   normalization.
"""

import sys

sys.path.insert(0, "/opt/trn_rl_repo")

import numpy as np

import concourse.bass as bass  # noqa: F401  (import registers engine classes)
import concourse.mybir as mybir
import concourse.tile as tile
from concourse import bacc
from concourse.bass_utils import run_bass_kernel_spmd

F16 = mybir.dt.float16
F32 = mybir.dt.float32

B, T, D = 4, 2048, 1024
N_HEAD, D_HEAD = 16, 64
ROPE_BASE = 10000.0
N_CORES = 8
HPC = N_HEAD // 2  # heads per core (8)
NPAIR = HPC // 2  # head pairs per core (4)
NKC = D // 128  # k-chunks (8)
NQC = T // 512  # q chunks of 512 (4)
NKB = T // 128  # k blocks of 128 (16)

SWAP_MASK = list(range(16, 32)) + list(range(0, 16))


def _build_program():
    nc = bacc.Bacc("TRN2", target_bir_lowering=False, debug=False,
                   num_devices=N_CORES)

    # host pre-packs every input in the exact SBUF free-layout so each
    # dram tensor loads with ONE fully-contiguous DMA.
    xT_d = nc.dram_tensor("xTp", [4, 128, NKC, 512], F16,
                          kind="ExternalInput").ap()
    wqk_d = nc.dram_tensor("wqkp", [128, NKC, 1024], F16,
                           kind="ExternalInput").ap()
    wv_d = nc.dram_tensor("wvp", [128, NKC, 512], F16,
                          kind="ExternalInput").ap()
    wp_d = nc.dram_tensor("wpp", [128, NPAIR, 1024], F16,
                          kind="ExternalInput").ap()
    cos_d = nc.dram_tensor("cos", [128, T], F16, kind="ExternalInput").ap()
    sin_d = nc.dram_tensor("sin", [128, T], F16, kind="ExternalInput").ap()
    outB_d = nc.dram_tensor("outB", [8, NQC, 128, 512], F16,
                            kind="ExternalOutput").ap()

    with tile.TileContext(nc) as tc:
        with tc.tile_pool(name="const", bufs=1) as cpool, \
             tc.tile_pool(name="big", bufs=1) as big, \
             tc.tile_pool(name="rope", bufs=2) as rope, \
             tc.tile_pool(name="pbuf", bufs=8) as pbuf, \
             tc.tile_pool(name="rbuf", bufs=2) as rbuf, \
             tc.tile_pool(name="ost", bufs=3) as ost:

            # ---- weights & tables: one contiguous DMA per tensor, in
            # first-use order (A-v needs wv + xT[tq0]; aqk(0,0) needs wqk,
            # xT[tq0..1], cos/sin; wp is last-used) ----
            wv_sb = big.tile([128, NKC, 512], F16)
            nc.sync.dma_start(wv_sb[:], wv_d)
            xT_sb = big.tile([128, 4, NKC, 512], F16)  # [p, tq, kc, 512]
            nc.sync.dma_start(xT_sb[:, 0], xT_d[0])
            nc.sync.dma_start(xT_sb[:, 1], xT_d[1])
            wqk_sb = big.tile([128, NKC, 1024], F16)
            nc.sync.dma_start(wqk_sb[:], wqk_d)
            cos_sb = cpool.tile([128, T], F16)
            nc.sync.dma_start(cos_sb[:], cos_d)
            sin_sb = cpool.tile([128, T], F16)
            nc.sync.dma_start(sin_sb[:], sin_d)
            nc.sync.dma_start(xT_sb[:, 2], xT_d[2])
            nc.sync.dma_start(xT_sb[:, 3], xT_d[3])
            wp_sb = big.tile([128, NPAIR, 1024], F16)
            nc.sync.dma_start(wp_sb[:], wp_d)

            def xT(kc, t0, t1):  # [128, t1-t0] view at 512-aligned offsets
                tq, r0 = divmod(t0, 512)
                assert t1 - t0 <= 512 - r0
                return xT_sb[:, tq, kc, r0:r0 + (t1 - t0)]

            v_aug = big.tile([128, NKB, HPC, 65], F16)
            nc.vector.memset(v_aug[:, :, :, 64:65], 1.0)  # ones column only

            qkT_sb = big.tile([128, 2 * NPAIR, T], F16)
            y_all = big.tile([128, NPAIR, T], F16)

            # ---- phase A: psum pools scoped so B/D pools fit in 8 banks ----
            phase_a = tc.tile_pool(name="vps", bufs=2, space="PSUM")
            vpsp = phase_a.__enter__()
            phase_a2 = tc.tile_pool(name="qkps", bufs=1, space="PSUM", side="right")
            qkpsp = phase_a2.__enter__()

            # HAM warmup: dummy matmuls on a zero tile while the first input
            # DMAs land.  The PE's clock gate needs ~3.4us of sustained
            # activity to lift 1.2GHz -> 2.4GHz; burning the DMA-wait time
            # on throwaway matmuls means phase A-v starts at full clock.
            warm = cpool.tile([128, 512], F16)
            nc.vector.memset(warm[:], 0.0)
            # preload the exp table-set too (2.7us, otherwise paid by the
            # first real exp right when attention starts)
            wexp = cpool.tile([1, 16], F16)
            nc.scalar.activation(wexp[:], warm[0:1, 0:16],
                                 mybir.ActivationFunctionType.Exp, scale=0.0)
            wps = vpsp.tile([128, 512], F32, name="warm_ps", tag="vps")
            for i in range(26):
                nc.tensor.matmul(wps[:], lhsT=warm[:, 0:128], rhs=warm[:],
                                 start=(i == 0), stop=(i == 25))

            # ---- phase A-v: v in natural layout [t, d] per 128-row block.
            # The first two aqk units interleave into the tail of this loop
            # so their RoPE chains (DVE) overlap the last A-v matmuls and
            # pair 0's scores can start the moment A-v ends. ----
            first_aqk = None  # populated after aqk_steps is defined

            def phase_av():
                steps = list(first_aqk)
                for tt in range(NKB):
                    vps_t = vpsp.tile([128, 512], F32,
                                      name=f"vps_{tt}", tag="vps")
                    for kc in range(NKC):
                        nc.tensor.matmul(
                            vps_t[:],
                            lhsT=xT(kc, tt * 128, (tt + 1) * 128),
                            rhs=wv_sb[:, kc, :],
                            start=(kc == 0), stop=(kc == NKC - 1),
                        )
                    nc.vector.tensor_copy(
                        v_aug[:, tt, :, 0:64],
                        vps_t[:].rearrange("p (h d) -> p h d", h=HPC),
                    )
                    if tt >= 7:
                        for _ in range(2):
                            if steps:
                                steps.pop(0)()
                while steps:
                    steps.pop(0)()

            # ---- phase A-qk units: one (ctile, T-half) projection+RoPE ----
            # ctile i<4 holds q of head pair i; ctile 4+i holds k of pair i.
            # Units are emitted interleaved into phase B so the PE always has
            # dense matmul work while ACT crunches exps (keeps HAM at 8/8).
            def aqk_steps(ct, hf):
                """Micro-steps for one aqk unit: 8 steps of 2 matmuls each
                (the 2x8-kc accumulation), then one DVE RoPE-chain step.
                Emitted a couple of steps per kb so the PE filler is smooth
                instead of a 3.4us burst."""
                box = {}

                def mk_mm(tcc, kcp):
                    def step():
                        if (tcc, kcp) == (0, 0):
                            box['ps'] = qkpsp.tile([128, 1024], F32,
                                                   name=f"qkps_{ct}_{hf}",
                                                   tag="qkps")
                        qkps_t = box['ps']
                        for kc in (2 * kcp, 2 * kcp + 1):
                            nc.tensor.matmul(
                                qkps_t[:, tcc * 512:(tcc + 1) * 512],
                                lhsT=wqk_sb[:, kc, ct * 128:(ct + 1) * 128],
                                rhs=xT(kc, hf * 1024 + tcc * 512,
                                       hf * 1024 + (tcc + 1) * 512),
                                start=(kc == 0), stop=(kc == NKC - 1),
                            )
                    return step

                def rope_step():
                    qkps_t = box['ps']
                    csl = slice(hf * 1024, (hf + 1) * 1024)
                    xbf = rope.tile([128, 1024], F16,
                                    name=f"xbf_{ct}_{hf}", tag="xbf")
                    nc.vector.tensor_copy(xbf[:], qkps_t[:])
                    ybf = rope.tile([128, 1024], F16,
                                    name=f"ybf_{ct}_{hf}", tag="ybf")
                    nc.vector.stream_shuffle(ybf[:], xbf[:], SWAP_MASK)
                    t1 = rope.tile([128, 1024], F16,
                                   name=f"t1_{ct}_{hf}", tag="t1")
                    nc.vector.tensor_tensor(t1[:], xbf[:], cos_sb[:, csl],
                                            mybir.AluOpType.mult)
                    t2 = rope.tile([128, 1024], F16,
                                   name=f"t2_{ct}_{hf}", tag="t2")
                    nc.vector.tensor_tensor(t2[:], ybf[:], sin_sb[:, csl],
                                            mybir.AluOpType.mult)
                    nc.vector.tensor_add(qkT_sb[:, ct, csl], t1[:], t2[:])

                return [mk_mm(tcc, kcp) for tcc in range(2)
                        for kcp in range(4)] + [rope_step]

            first_aqk = aqk_steps(0, 0) + aqk_steps(4, 0)
            phase_av()

            phase_a.__exit__(None, None, None)  # close vps pool

            phase_b2 = tc.tile_pool(name="ytps", bufs=2, space="PSUM")
            ytpsp = phase_b2.__enter__()
            phase_b = tc.tile_pool(name="sps", bufs=2, space="PSUM")
            spsp = phase_b.__enter__()

            opsp = None

            def d_ot_steps(qc, ot):
                """Micro-steps for one out-proj unit: 4 single matmuls
                (pair accumulation, pair 3 last) + evacuate/DMA step."""
                box = {}

                def mk_mm(pr):
                    def step():
                        if pr == 0:
                            box['ps'] = opsp.tile([128, 512], F32,
                                                  name=f"ops_{qc}_{ot}",
                                                  tag="ops")
                        nc.tensor.matmul(
                            box['ps'][:],
                            lhsT=wp_sb[:, pr, ot * 128:(ot + 1) * 128],
                            rhs=y_all[:, pr, qc * 512:(qc + 1) * 512],
                            start=(pr == 0), stop=(pr == NPAIR - 1),
                        )
                    return step

                def evac_step():
                    st = ost.tile([128, 512], F16,
                                  name=f"st_{qc}_{ot}", tag="st")
                    nc.vector.tensor_copy(st[:], box['ps'][:])
                    nc.sync.dma_start(outB_d[ot, qc], st[:])

                return [mk_mm(pr) for pr in range(NPAIR)] + [evac_step]

            # ---- filler pacing: aqk units (then out-proj units) are fed
            # into the kb loop a couple of micro-steps at a time, against
            # per-unit deadlines (global kb index when first consumed) ----
            AQK_ORDER = [(0, 1), (4, 1), (1, 0), (5, 0), (1, 1), (5, 1),
                         (2, 0), (6, 0), (2, 1), (6, 1),
                         (3, 0), (7, 0), (3, 1), (7, 1)]
            AQK_DEADLINE = {(0, 1): 12, (4, 1): 12,
                            (1, 0): 38, (5, 0): 38, (1, 1): 38, (5, 1): 38,
                            (2, 0): 78, (6, 0): 78, (2, 1): 78, (6, 1): 78,
                            (3, 0): 116, (7, 0): 116,
                            (3, 1): 116, (7, 1): 116}
            aqk_queue = []
            for u in AQK_ORDER:
                aqk_queue.extend(aqk_steps(*u))
            # cumulative step targets at each deadline (9 steps/unit)
            AQK_CHECK = [(0, 0), (12, 18), (38, 54), (78, 90), (116, 126)]

            def aqk_target(g):
                for (g0, c0), (g1, c1) in zip(AQK_CHECK, AQK_CHECK[1:]):
                    if g <= g1:
                        return c0 + (c1 - c0) * (g - g0) / (g1 - g0)
                return len(aqk_queue)

            aqk_pos_box = [0]

            def pump_aqk(g):
                """Emit queued aqk steps, linearly paced so each unit lands
                just before its deadline — a smooth ~1-1.5 step/kb trickle
                of PE filler instead of 3.4us bursts."""
                import math
                want = min(len(aqk_queue), math.ceil(aqk_target(g + 1)))
                while aqk_pos_box[0] < want:
                    aqk_queue[aqk_pos_box[0]]()
                    aqk_pos_box[0] += 1

            dfill = []  # pending d_ot micro-steps (flat)

            # ---- phase B: attention in scoresT layout, pair-outer.
            # Per kb the Tensor-queue order is: scores(kb) ... PV(kb-1), so
            # the PE streams scores for the next block while ACT exps the
            # previous one (1-deep software pipeline; sps bufs=2 holds both).
            gkb = 0  # global kb counter (0..159)
            for pr in range(NPAIR):
                q_t = qkT_sb[:, pr, :]
                k_t = qkT_sb[:, NPAIR + pr, :]
                for qc in range(NQC):
                    if pr == 3 and qc == 0:
                        # drain any aqk remainder, close its PSUM pool, and
                        # open the out-proj pool in the freed banks
                        while aqk_pos_box[0] < len(aqk_queue):
                            aqk_queue[aqk_pos_box[0]][1]()
                            aqk_pos_box[0] += 1
                        phase_a2.__exit__(None, None, None)
                        phase_b3 = tc.tile_pool(name="ops", bufs=2, space="PSUM", side="right")
                        opsp = phase_b3.__enter__()
                    yt0 = ytpsp.tile([65, 512], F32,
                                     name=f"yt0_{qc}_{pr}", tag="yt")
                    yt1 = ytpsp.tile([65, 512], F32,
                                     name=f"yt1_{qc}_{pr}", tag="yt")
                    yts = (yt0, yt1)
                    nkb = 4 * qc + 4
                    pend = []  # (kb, pt, off) whose PV is not yet emitted

                    def emit_pv(kb, pt, off):
                        for h in range(2):
                            nc.tensor.matmul(
                                yts[h][:, off:512],
                                lhsT=v_aug[:, kb, 2 * pr + h, :],
                                rhs=pt[:, h, off:512],
                                start=(kb == 0), stop=(kb == nkb - 1),
                                skip_group_check=True,
                            )

                    # kb's are processed in PAIRS on the PE: scores(2j),
                    # scores(2j+1) back-to-back (same 64-row array config),
                    # then PV(2j-2),PV(2j-1) and the filler matmuls (128-row
                    # configs).  This halves the ~100ns row-reconfig penalty
                    # the PE pays between 64-row and 128-row matmuls.
                    for kb in range(nkb):
                        off = max(0, (kb - 4 * qc) * 128)
                        sps_t = spsp.tile([128, 2, 512], F32,
                                          name=f"sps_{qc}_{pr}_{kb}", tag="sps")
                        for h in range(2):
                            nc.tensor.matmul(
                                sps_t[:, h, off:512],
                                lhsT=k_t[h * 64:(h + 1) * 64,
                                         kb * 128:(kb + 1) * 128],
                                rhs=q_t[h * 64:(h + 1) * 64,
                                        qc * 512 + off:(qc + 1) * 512],
                                start=True, stop=True,
                            )
                        pt = pbuf.tile([128, 2, 512], F16,
                                       name=f"pt_{qc}_{pr}_{kb}", tag="pt")
                        nc.scalar.activation(
                            pt[:, :, off:512], sps_t[:, :, off:512],
                            mybir.ActivationFunctionType.Exp, scale=0.125)
                        if kb >= 4 * qc:  # diagonal block: triangular mask
                            for h in range(2):
                                nc.gpsimd.affine_select(
                                    out=pt[:, h, off:off + 128],
                                    in_=pt[:, h, off:off + 128],
                                    compare_op=mybir.AluOpType.is_ge,
                                    fill=0.0, base=0,
                                    pattern=[[1, 128]],
                                    channel_multiplier=-1)
                        pend.append((kb, pt, off))
                        if kb % 2 == 1:
                            while len(pend) > 2:
                                emit_pv(*pend.pop(0))
                            if pr < 3:
                                pump_aqk(gkb)
                                pump_aqk(gkb + 1)
                            elif kb >= 3:
                                # ride out-proj micro-steps; delayed so the
                                # previous qc's normalization (y_all) is done
                                for _ in range(8):
                                    if dfill:
                                        dfill.pop(0)()
                        gkb += 1
                    while pend:
                        emit_pv(*pend.pop(0))
                    # Normalize y/l off the PE critical path.  The four
                    # yt-releasing copies run FIRST on DVE (frees both PSUM
                    # slots ~1.4us after PV so the next qc's accumulation
                    # never stalls); the trailing chain (1/l on DVE,
                    # partition-broadcast on GPSIMD, in-place multiply on
                    # DVE) only gates the pr==3 out-proj riders.  For the
                    # very last group (pr3,qc3) nothing accumulates next, so
                    # skip the y copies and fold them into the multiply to
                    # shorten the serial tail.
                    last = (pr == 3 and qc == 3)
                    ysls = [y_all[h * 64:(h + 1) * 64, pr,
                                  qc * 512:(qc + 1) * 512] for h in range(2)]
                    lsbs = []
                    if not last:
                        for h in range(2):
                            nc.vector.tensor_copy(ysls[h], yts[h][0:64, :])
                    for h in range(2):
                        lsb = rbuf.tile([1, 512], F32,
                                        name=f"lsb_{qc}_{pr}_{h}", tag="lsb")
                        nc.vector.tensor_copy(lsb[:], yts[h][64:65, :])
                        lsbs.append(lsb)
                    rb = rbuf.tile([128, 512], F32,
                                   name=f"rb_{qc}_{pr}", tag="rb")
                    rrs = []
                    for h in range(2):
                        rr = rbuf.tile([1, 512], F32,
                                       name=f"rr_{qc}_{pr}_{h}", tag="rr")
                        nc.vector.reciprocal_approx_fast(rr[:], lsbs[h][:])
                        rrs.append(rr)
                    # both broadcasts write at base partition 0 (the Q7
                    # kernel mishandles non-zero output bases): h1 fills all
                    # 128 partitions, then h0 overwrites the low 64.
                    nc.gpsimd.partition_broadcast(rb[:], rrs[1][:],
                                                  channels=128)
                    nc.gpsimd.partition_broadcast(rb[0:64, :], rrs[0][:],
                                                  channels=64)
                    for h in (1, 0):
                        if last:
                            nc.vector.tensor_tensor(
                                ysls[h], yts[h][0:64, :],
                                rb[h * 64:(h + 1) * 64, :],
                                mybir.AluOpType.mult)
                        else:
                            nc.vector.tensor_tensor(
                                ysls[h], ysls[h],
                                rb[h * 64:(h + 1) * 64, :],
                                mybir.AluOpType.mult)
                    if pr == 3:
                        for ot in range(8):
                            dfill.extend(d_ot_steps(qc, ot))

            # ---- tail: qc3's eight out-proj units remain.  Drain them
            # pair-staggered on the existing 2-deep ops pool (no PSUM pool
            # swap: its DRAIN barrier would block the tail matmuls): each
            # pair's six pair0-2 accumulations issue immediately after the
            # PV flush and cover the final normalization chain, which only
            # the pair-3 matmuls wait on. ----
            units = [dfill[i * 5:(i + 1) * 5] for i in range(len(dfill) // 5)]
            dfill.clear()
            for u0, u1 in zip(units[0::2], units[1::2]):
                for s in (u0[0], u0[1], u0[2], u1[0], u1[1], u1[2],
                          u0[3], u0[4], u1[3], u1[4]):
                    s()

            phase_b.__exit__(None, None, None)
            phase_b3.__exit__(None, None, None)
            phase_b2.__exit__(None, None, None)

    nc.compile()
    return nc


def _host_inputs(x, W_attn, W_proj):
    """Build the per-core input maps (host-side shard + layout prep)."""
    j = np.arange(16)
    perm = np.concatenate([2 * j, 2 * j + 1, 32 + 2 * j, 33 + 2 * j])

    # RoPE tables in the permuted-transposed layout, fp32 math then fp16.
    inv_freq = 1.0 / (ROPE_BASE ** (np.arange(0, D_HEAD, 2, dtype=np.float64)
                                    / D_HEAD))  # [32]
    t = np.arange(T, dtype=np.float64)
    freqs = np.outer(inv_freq, t)  # [32, T]
    jmap = np.concatenate([j, j, 16 + j, 16 + j])  # per-head 64 rows
    jmap = np.concatenate([jmap, jmap])  # 128 rows (2 heads)
    sign = np.tile(np.concatenate([-np.ones(16), np.ones(16)]), 4)  # [128]
    cos_tab = np.ascontiguousarray(np.cos(freqs[jmap]).astype(np.float16))
    sin_tab = np.ascontiguousarray(
        (sign[:, None] * np.sin(freqs[jmap])).astype(np.float16))

    def pack_kc(w):  # [D, n] -> [128, D//128, n] partition-major contiguous
        n = w.shape[1]
        return np.ascontiguousarray(
            w.reshape(-1, 128, n).transpose(1, 0, 2))

    in_maps = []
    for c in range(N_CORES):
        b, half = divmod(c, 2)
        heads = [8 * half + i for i in range(HPC)]
        # wqk: 4 q-pair ctiles then 4 k-pair ctiles, per-head perm'd cols
        cols = []
        for base in (0, D):  # q block, k block of W_attn
            for hp in range(NPAIR):
                for g in (heads[2 * hp], heads[2 * hp + 1]):
                    cols.append(base + g * D_HEAD + perm)
        wqk = pack_kc(W_attn[:, np.concatenate(cols)].astype(np.float16))
        wv = pack_kc(W_attn[:, 2 * D + 512 * half: 2 * D + 512 * (half + 1)]
                     .astype(np.float16))
        wp = pack_kc(W_proj[512 * half: 512 * (half + 1), :]
                     .astype(np.float16))
        xT = x[b].T.astype(np.float16)  # [D, T]
        # [tq, 128, kc, 512]: xTp[tq, p, kc, n] = xT[kc*128+p, tq*512+n]
        xTp = np.ascontiguousarray(
            xT.reshape(NKC, 128, 4, 512).transpose(2, 1, 0, 3))
        in_maps.append({
            "xTp": xTp, "wqkp": wqk, "wvp": wv, "wpp": wp,
            "cos": cos_tab, "sin": sin_tab,
        })
    return in_maps


_NC_CACHE = None


def kernel(x, W_attn, W_proj, _trace=False):
    global _NC_CACHE
    x = np.asarray(x, dtype=np.float32)
    W_attn = np.asarray(W_attn, dtype=np.float32)
    W_proj = np.asarray(W_proj, dtype=np.float32)

    if _NC_CACHE is None:
        _NC_CACHE = _build_program()
    nc = _NC_CACHE

    in_maps = _host_inputs(x, W_attn, W_proj)
    res = run_bass_kernel_spmd(nc, in_maps, core_ids=list(range(N_CORES)),
                               trace=_trace)

    y = np.empty((B, T, D), dtype=np.float32)
    for b in range(B):
        # outB [8(ot), 4(qc), 128, 512]: outT[ot*128+p, qc*512+n]
        s = (res.results[2 * b]["outB"].astype(np.float32)
             + res.results[2 * b + 1]["outB"].astype(np.float32))
        y[b] = s.transpose(1, 3, 0, 2).reshape(T, D)
    if _trace:
        return y, res
    return y
